# revision 1
# baseline (speedup 1.0000x reference)
"""Trainium2 Bass kernel for the CN coupling-block problem (nn_CN_69312182223156).

Math (per subnet s on half-features x_s with conditioner c):
    h   = relu(c @ W1 + b1)                       # [B, 50]
    p   = h @ W2 + b2                             # [B, 9696]
    m1, b1p, m2 = p[:, :3200], p[:, 3200:6400], p[:, 6400:9600]   (viewed [B,32,100])
    bias2, eps, alpha = p[:, 9600:9632], p[:, 9632:9664]/10, p[:, 9664:]/10
    z   = x*m1 + b1p
    num = sum_l elu(z)*m2 ;  den = sum_l relu(-m1*m2) + 1
    y   = exp(alpha) * (x + 0.8*sigmoid(eps)*num/den) + bias2

Subnet 1: x=x1, c=x2.  Subnet 2: x=x2, c=y1.  Output concat([y1, y2]).

Strategy: pure data-parallel over 8 cores (2048 rows each), weights replicated.
Batch rows on SBUF partitions (tiles of 128). Biases folded into augmented
weights; b1 region carries b1+1 so z1 = z+1 and elu(z)+1 = max(z1, exp(z1-1))
with exp argument clamped via zn = min(z1, 1).
Engine split (4-way balance):
  PE  : all matmuls (m1/b1/m2 in 400-col PSUM-bank halves, tails, hT).
  Pool: zn = min(z1,1) const tensor_scalar, b1s/m1s PSUM->SBUF f16 copies.
  ACT : exp, m2s copy, rden = relu(-u) (scale=-1), part of zmul via Copy+scale.
  DVE : zmul (per-dim tensor_scalar 4x), zadd, w = max(z1,e), t = w*m2,
        u = m1*m2, per-tile fold tree for the l-reduction, tail arithmetic.
num = sum_l w*m2 - S2 (S2 = sum_l m2 appended as 32 extra weight columns).
"""

import numpy as np

B = 16384
DIM = 32
LS = 100
NCORES = 8
BC = B // NCORES          # rows per core
NT = BC // 128            # 128-row tiles per core
DL = DIM * LS             # 3200
PW = 3 * DL + 3 * DIM     # 9696 params per row
CHUNK = 800               # params per elementwise chunk (8 dims x 100)
HALF = 400                # params per PSUM-bank matmul
NCHUNK = DL // CHUNK      # 4
DPC = CHUNK // LS         # 8 dims per chunk

# ---- engine-balance knobs (tuned against TimelineSim) ----
ZMUL_ACT_DIMS = 2
E_BIT_A = 1477.3196
E_BIT_B = 13823.9  # 15360 - A - 59.3 (Schraudolph corr) + 0.5 (trunc comp)         # of the 8 dims per chunk, this many run on ACT
RDEN_ACT_MOD = 1          # rden on ACT when (chunkidx % RDEN_ACT_MOD*2 < RDEN_ACT_MOD)... see code
M2S_ON_ACT = True
M1S_ON_POOL = False
B1S_ON_POOL = False
ZN_ON_POOL = True

_cache = {}


def _build_program():
    import concourse.bass as bass
    import concourse.tile as tile
    import concourse.mybir as mybir
    from concourse import bacc, masks

    f32 = mybir.dt.float32
    f16 = mybir.dt.float16
    Alu = mybir.AluOpType
    Act = mybir.ActivationFunctionType
    X = mybir.AxisListType.X

    nc = bacc.Bacc("TRN2", target_bir_lowering=False)

    x_d = nc.dram_tensor("x", [BC, 2 * DIM], f32, kind="ExternalInput")
    w1a = [nc.dram_tensor(f"w1a{s}", [DIM + 1, 51], f16, kind="ExternalInput")
           for s in (1, 2)]
    w2a = [nc.dram_tensor(f"w2a{s}", [51, PW + DIM], f16, kind="ExternalInput")
           for s in (1, 2)]
    y_d = nc.dram_tensor("y", [BC, 2 * DIM], f32, kind="ExternalOutput")

    with tile.TileContext(nc) as tc:
        with (
            tc.tile_pool(name="const", bufs=1) as const,
            tc.tile_pool(name="per", bufs=1) as per,
            tc.tile_pool(name="mid", bufs=4) as mid,
            tc.tile_pool(name="ew", bufs=6) as ew,
            tc.tile_pool(name="big", bufs=3) as bigp,
            tc.tile_pool(name="fold", bufs=2) as foldp,
            tc.tile_pool(name="tail", bufs=3) as tailp,
            tc.tile_pool(name="pmm", bufs=3, space="PSUM") as pmm,
            tc.tile_pool(name="psm", bufs=2, space="PSUM") as psm,
        ):
            # ---- constants ----
            w1s = []
            w2s = []
            for s in range(2):
                t1 = const.tile([DIM + 1, 51], f16, tag=f"w1_{s}")
                nc.sync.dma_start(t1, w1a[s][:])
                w1s.append(t1)
                t2 = const.tile([51, PW + DIM], f16, tag=f"w2_{s}")
                nc.sync.dma_start(t2, w2a[s][:])
                w2s.append(t2)
            identf = const.tile([128, 128], f32, tag="identf")
            masks.make_identity(nc, identf[:])
            negone = const.tile([128, 1], f32, tag="negone")
            nc.vector.memset(negone, -1.0)

            def subnet(s, it, xf, condT, y_out):
                # h^T = relu(W1^T c^T + b1): [51, 128]; col 50 of W1aug is
                # e_32 so row 50 comes out as relu(1) = 1 (the aug ones row).
                h_ps = psm.tile([51, 128], f32, tag="tp")
                nc.tensor.matmul(h_ps, w1s[s], condT, start=True, stop=True)
                hT = mid.tile([51, 128], f16, tag="hT")
                nc.scalar.activation(hT, h_ps, Act.Relu)

                xc32 = xf[:, s * DIM:(s + 1) * DIM]   # f32 x for this subnet
                big = bigp.tile([128, 2, DIM, LS], f16, tag="big")  # t | r

                for c in range(NCHUNK):
                    ci = it * NCHUNK + c
                    base = c * 3 * CHUNK
                    # -- PE: m1 | b1p | m2 into rotating PSUM slots --
                    m1p = pmm.tile([128, 2, 512], f32, tag="mm")
                    for hh in range(2):
                        o = base + hh * HALF
                        nc.tensor.matmul(m1p[:, hh, 0:HALF], hT,
                                         w2s[s][:, o:o + HALF],
                                         start=True, stop=True)
                    m1s = ew.tile([128, CHUNK], f16, tag="m1s")
                    m1s2 = m1s.rearrange("p (h q) -> p h q", h=2)
                    if M1S_ON_POOL:
                        nc.gpsimd.tensor_copy(m1s2, m1p[:, :, 0:HALF])
                    else:
                        nc.scalar.copy(m1s2, m1p[:, :, 0:HALF])

                    b1p = pmm.tile([128, 2, 512], f32, tag="mm")
                    for hh in range(2):
                        o = base + CHUNK + hh * HALF
                        nc.tensor.matmul(b1p[:, hh, 0:HALF], hT,
                                         w2s[s][:, o:o + HALF],
                                         start=True, stop=True)
                    b1s = ew.tile([128, CHUNK], f16, tag="b1s")
                    b1s2 = b1s.rearrange("p (h q) -> p h q", h=2)
                    if B1S_ON_POOL:
                        nc.gpsimd.tensor_copy(b1s2, b1p[:, :, 0:HALF])
                    else:
                        nc.scalar.copy(b1s2, b1p[:, :, 0:HALF])

                    m2p = pmm.tile([128, 2, 512], f32, tag="mm")
                    for hh in range(2):
                        o = base + 2 * CHUNK + hh * HALF
                        nc.tensor.matmul(m2p[:, hh, 0:HALF], hT,
                                         w2s[s][:, o:o + HALF],
                                         start=True, stop=True)
                    m2s = ew.tile([128, CHUNK], f16, tag="m2s")
                    m2s2 = m2s.rearrange("p (h q) -> p h q", h=2)
                    if M2S_ON_ACT:
                        nc.scalar.copy(m2s2, m2p[:, :, 0:HALF])
                    else:
                        nc.gpsimd.tensor_copy(m2s2, m2p[:, :, 0:HALF])

                    # -- z = x*m1 (per-dim broadcast), split DVE/ACT --
                    zm = ew.tile([128, CHUNK], f16, tag="zm")
                    zm3 = zm.rearrange("p (d l) -> p d l", l=LS)
                    m1s3 = m1s.rearrange("p (d l) -> p d l", l=LS)
                    for j in range(DPC):
                        xj = xc32[:, c * DPC + j:c * DPC + j + 1]
                        if j >= DPC - ZMUL_ACT_DIMS:
                            nc.scalar.activation(zm3[:, j, :], m1s3[:, j, :],
                                                 Act.Copy, scale=xj)
                        else:
                            nc.vector.tensor_scalar_mul(
                                zm3[:, j, :], m1s3[:, j, :], xj)
                    # z1 = zm + (b1+1)
                    z1 = ew.tile([128, CHUNK], f16, tag="z1")
                    nc.vector.tensor_add(z1, zm, b1s)
                    # zn = min(z1, 1); e = exp(zn-1) via f16 bit-trick:
                    # int16(A*zn + B) reinterpreted as f16 (Schraudolph).
                    zn = ew.tile([128, CHUNK], f16, tag="zn")
                    nc.gpsimd.tensor_scalar_min(zn, z1, 1.0)
                    e2 = ew.tile([128, CHUNK], mybir.dt.int16, tag="e2")
                    nc.gpsimd.tensor_scalar(e2, zn, E_BIT_A, E_BIT_B,
                                            Alu.mult, Alu.add)
                    # w = max(z1, e) = elu(z) + 1
                    w = ew.tile([128, CHUNK], f16, tag="w")
                    nc.vector.tensor_tensor(w, z1, e2[:, :].bitcast(f16), Alu.max)
                    # t = w*m2 into big[:,0]; r = relu(-m1*m2) into big[:,1]
                    tdst = big[:, 0, c * DPC:(c + 1) * DPC, :]
                    nc.vector.tensor_mul(tdst, w.rearrange("p (d l) -> p d l", l=LS), m2s.rearrange("p (d l) -> p d l", l=LS))
                    u = ew.tile([128, CHUNK], f16, tag="u")
                    nc.vector.tensor_mul(u, m1s, m2s)
                    rdst = big[:, 1, c * DPC:(c + 1) * DPC, :]
                    u3 = u.rearrange("p (d l) -> p d l", l=LS)
                    nc.gpsimd.tensor_scalar(rdst, u3, -1.0, 0.0,
                                            Alu.mult, Alu.max)

                # ---- fold tree over l: [128,2,32,100] -> [128,2,32] f32 ----
                n1 = foldp.tile([128, 2, DIM, 50], f16, tag="n1")
                nc.vector.tensor_add(n1, big[:, :, :, 0:50], big[:, :, :, 50:100])
                n2 = foldp.tile([128, 2, DIM, 25], f16, tag="n2")
                nc.vector.tensor_add(n2, n1[:, :, :, 0:25], n1[:, :, :, 25:50])
                n3 = foldp.tile([128, 2, DIM, 12], f16, tag="n3")
                nc.vector.tensor_add(n3, n2[:, :, :, 0:12], n2[:, :, :, 12:24])
                n4 = foldp.tile([128, 2, DIM, 6], f16, tag="n4")
                nc.vector.tensor_add(n4, n3[:, :, :, 0:6], n3[:, :, :, 6:12])
                n5 = foldp.tile([128, 2, DIM, 3], f16, tag="n5")
                nc.vector.tensor_add(n5, n4[:, :, :, 0:3], n4[:, :, :, 3:6])
                s1 = foldp.tile([128, 2, DIM], f16, tag="s1")
                nc.vector.tensor_add(s1, n5[:, :, :, 0], n5[:, :, :, 1])
                s2 = foldp.tile([128, 2, DIM], f16, tag="s2")
                nc.vector.tensor_add(s2, n5[:, :, :, 2], n2[:, :, :, 24])
                numden = tailp.tile([128, 2, DIM], f32, tag="numden")
                nc.vector.tensor_add(numden, s1, s2)

                # ---- tail (bias2 | eps | alpha | S2) ----
                tp = psm.tile([128, 4 * DIM], f32, tag="tp")
                nc.tensor.matmul(tp, hT, w2s[s][:, 3 * DL:3 * DL + 4 * DIM],
                                 start=True, stop=True)
                b2p = tp[:, 0:DIM]
                epp = tp[:, DIM:2 * DIM]
                alp = tp[:, 2 * DIM:3 * DIM]
                s2p = tp[:, 3 * DIM:4 * DIM]

                den = tailp.tile([128, DIM], f32, tag="den")
                nc.gpsimd.tensor_scalar_add(den, numden[:, 1, :], 1.0)
                rec = tailp.tile([128, DIM], f32, tag="rec")
                nc.vector.reciprocal_approx_fast(rec, den)
                # sigmoid(eps/10) = 1 / (1 + exp(-eps/10))
                nege = tailp.tile([128, DIM], f32, tag="nege")
                nc.scalar.activation(nege, epp, Act.Exp, scale=-0.1)
                sd = tailp.tile([128, DIM], f32, tag="sd")
                nc.gpsimd.tensor_scalar_add(sd, nege, 1.0)
                sig = tailp.tile([128, DIM], f32, tag="sig")
                nc.vector.reciprocal_approx_fast(sig, sd)
                ea = tailp.tile([128, DIM], f32, tag="ea")
                nc.scalar.activation(ea, alp, Act.Exp, scale=0.1)
                nums = tailp.tile([128, DIM], f32, tag="nums")
                nc.vector.tensor_sub(nums, numden[:, 0, :], s2p)
                frac = tailp.tile([128, DIM], f32, tag="frac")
                nc.vector.tensor_mul(frac, nums, rec)
                q = tailp.tile([128, DIM], f32, tag="q")
                nc.vector.scalar_tensor_tensor(
                    q, in0=frac, scalar=0.8, in1=sig, op0=Alu.mult, op1=Alu.mult)
                sx = tailp.tile([128, DIM], f32, tag="sx")
                nc.vector.tensor_add(sx, q, xc32)
                yp = tailp.tile([128, DIM], f32, tag="yp")
                nc.vector.tensor_mul(yp, ea, sx)
                nc.vector.tensor_add(y_out[:, s * DIM:(s + 1) * DIM], yp, b2p)

            xfs, youts = {}, {}
            for it in range(NT):
                r0 = it * 128
                xf = per.tile([128, 2 * DIM + 1], f32, tag=f"xf{it}")
                nc.sync.dma_start(xf[:, 0:2 * DIM], x_d[r0:r0 + 128, :])
                nc.gpsimd.memset(xf[:, 2 * DIM:], 1.0)

                # conditioner for subnet 1: [x2 | 1]^T  -> [33, 128]
                ct_ps = psm.tile([DIM + 1, 128], f32, tag="tp")
                nc.tensor.transpose(ct_ps, xf[:, DIM:2 * DIM + 1], identf)
                condT = mid.tile([DIM + 1, 128], f16, tag="condT")
                nc.scalar.copy(condT, ct_ps)

                y_out = per.tile([128, 2 * DIM], f32, tag=f"y_out{it}")
                subnet(0, it, xf, condT, y_out)
                xfs[it], youts[it] = xf, y_out

            for it in range(NT):
                r0 = it * 128
                xf, y_out = xfs[it], youts[it]
                # conditioner for subnet 2: [y1 | 1]^T
                c2_ps = psm.tile([DIM, 128], f32, tag="tp")
                nc.tensor.transpose(c2_ps, y_out[:, 0:DIM], identf)
                condT2 = mid.tile([DIM + 1, 128], f16, tag="condT2")
                nc.scalar.copy(condT2[0:DIM, :], c2_ps)
                nc.gpsimd.memset(condT2[DIM:DIM + 1, :], 1.0)
                subnet(1, it, xf, condT2, y_out)
                nc.sync.dma_start(y_d[r0:r0 + 128, :], y_out)

    nc.compile()
    return nc


def _prep_weights(W1, b1, W2, b2):
    w1a = np.concatenate([W1, b1[None, :]], axis=0).astype(np.float16)  # [33, 50]
    ones_col = np.zeros((DIM + 1, 1), dtype=np.float16)
    ones_col[DIM, 0] = 1.0
    w1a = np.concatenate([w1a, ones_col], axis=1)                       # [33, 51]
    w2a = np.concatenate([W2, b2[None, :]], axis=0)                     # [51, 9696] f32
    w2a = w2a.copy()
    w2a[50, DL:2 * DL] += 1.0   # bias1 region delivers b1+1 (z1 = z+1)
    # append S2 columns: S2[:, d] = sum_l w2a[:, mat2 region (d, l)]
    m2cols = w2a[:, 2 * DL:3 * DL].reshape(51, DIM, LS)
    s2 = m2cols.sum(axis=2)                                             # [51, DIM]
    w2a = np.concatenate([w2a, s2], axis=1)                             # [51, 9728]
    # interleave chunk-blocks: [m1_c | b1_c | m2_c] x 4 chunks, tails last
    out = np.empty_like(w2a)
    for c in range(NCHUNK):
        src_m1 = w2a[:, c * CHUNK:(c + 1) * CHUNK]
        src_b1 = w2a[:, DL + c * CHUNK:DL + (c + 1) * CHUNK]
        src_m2 = w2a[:, 2 * DL + c * CHUNK:2 * DL + (c + 1) * CHUNK]
        base = c * 3 * CHUNK
        out[:, base:base + CHUNK] = src_m1
        out[:, base + CHUNK:base + 2 * CHUNK] = src_b1
        out[:, base + 2 * CHUNK:base + 3 * CHUNK] = src_m2
    out[:, 3 * DL:] = w2a[:, 3 * DL:]
    return (np.ascontiguousarray(w1a),
            np.ascontiguousarray(out.astype(np.float16)))


def kernel(**inputs):
    from concourse.bass_utils import run_bass_kernel_spmd

    if "nc" not in _cache:
        _cache["nc"] = _build_program()
    nc = _cache["nc"]

    x = np.ascontiguousarray(inputs["x"], dtype=np.float32)
    w1a1, w2a1 = _prep_weights(inputs["s1_W1"], inputs["s1_b1"],
                               inputs["s1_W2"], inputs["s1_b2"])
    w1a2, w2a2 = _prep_weights(inputs["s2_W1"], inputs["s2_b1"],
                               inputs["s2_W2"], inputs["s2_b2"])

    in_maps = []
    for i in range(NCORES):
        in_maps.append({
            "x": x[i * BC:(i + 1) * BC],
            "w1a1": w1a1, "w2a1": w2a1,
            "w1a2": w1a2, "w2a2": w2a2,
        })

    last_err = None
    for attempt in range(3):
        try:
            res = run_bass_kernel_spmd(nc, in_maps, core_ids=list(range(NCORES)),
                                       **_cache.get("run_kwargs", {}))
            out = np.concatenate([r["y"] for r in res.results], axis=0)
            _cache["last_results"] = res
            return out
        except Exception as ex:  # transient NRT/device errors: retry
            last_err = ex
    raise last_err

